# revision 23
# baseline (speedup 1.0000x reference)
"""Distributed Trainium2 kernel for AGGEdgeGraph message passing (v2).

Reference:
    t = edge_feats @ W + b                      # [E, H]
    out[i] = t[i] + sum_k t[neighbors[i, k]]    # [E, H]

Strategy (8 cores, edge-sharded, E/8 = 12500 -> padded EPC=12800/core):
  - Phase 1: per-core matmul t = x @ W (NO bias) -> bf16 -> ag_in DRAM.
  - AllGather ag_in -> table [8*EPC, H].  Gather index ranges are the 4
    table quarters (25600 rows < 32768, int16 dma_gather idx limit).
  - Tokens: for each edge window (128 edges) and source class
    (0 = local/own-shard incl. a "self" token per edge, 1..4 = table
    quarters), the window's tokens form a cell padded to x128.  Class
    streams are chopped into dma_gather calls of <=1024 idx, issued
    round-robin over DMA queues.  Pad tokens gather row 0 and carry
    dest=-1 so their selector row is all-zero; no memsets, no count regs.
  - Reduce: per window, psum[128e,128h] = onehot-row matmul (adds
    (K+1)*b to every edge) + sum over chunks of matmul(lhsT=S, rhs=g)
    where S[tok, e] = (dest[tok] == iota[e]) built on DVE via batched
    tensor_tensor is_equal (8 chunks per op).  Output copied psum->SBUF
    on the Scalar engine, then DMA'd out.
  - SPMD: same graph on all cores; per-core behavior only via input data
    (idx/dest arrays).  Cell sizes are max over cores.
"""

import sys

if "/opt/trn_rl_repo" not in sys.path:
    sys.path.insert(0, "/opt/trn_rl_repo")

import numpy as np
import ml_dtypes

import concourse.bass as bass
import concourse.bacc as bacc
import concourse.mybir as mybir
import concourse.tile as tile
from concourse.bass_utils import run_bass_kernel_spmd

NCORES = 8
F = 256
H = 128
K = 16

E_FULL = 100000
EPC_RAW = E_FULL // NCORES   # 12500
EPC = 12800                  # padded per-core edges (multiple of 128)
WIN = 128                    # edges per psum window
MAXCALL = 1024               # dma_gather idx cap (DGE ring limit)
NQ = 4                       # SWDGE queues used round-robin (ucode max 4)
NCLS = 5                     # 0 = local shard, 1..4 = table quarters

BF16 = mybir.dt.bfloat16
F32 = mybir.dt.float32
NP_BF16 = ml_dtypes.bfloat16


# ---------------------------------------------------------------------------
# Host-side planning
# ---------------------------------------------------------------------------

def plan(neighbors, epc=EPC, ncores=NCORES):
    """Build the shared call/cell structure and per-core packed arrays.

    Returns dict with:
      calls: list of (cls, num_idxs) in issue order (per class, stream order)
      cells: per window list of (cls, chunk_start, nchunks) in PE order
      tot16, totch
      per_core: list of dicts with 'idx' [128, tot16] int16 and
                'dst' [128, totch] f32
    """
    e_full = neighbors.shape[0]
    epc_raw = e_full // ncores
    nwin = epc // WIN
    CW = 4 * WIN                 # cell width in edges (4 windows per cell)
    ncell = epc // CW
    qrows = 2 * epc              # table quarter rows (2 shards per quarter)

    # ---- per-core token lists grouped by (cell, class) ----
    # token: (idx_in_range, dest_in_cell 0..CW-1)
    grouped = []   # [core][cell][cls] -> (idx int64 array, dest int64 array)
    for c in range(ncores):
        nb = np.asarray(neighbors[c * epc_raw:(c + 1) * epc_raw], np.int64)
        v = epc * (nb // epc_raw) + (nb % epc_raw)      # padded-global rows
        e = np.repeat(np.arange(epc_raw, dtype=np.int64), K)
        v = v.reshape(-1)
        w = e // CW
        dest = e % CW
        is_local = (v // epc) == c
        cls = np.where(is_local, 0, 1 + v // qrows)
        idxv = np.where(is_local, v - c * epc, v - (v // qrows) * qrows)
        order = np.lexsort((dest, cls, w))
        w_s, c_s, i_s, d_s = w[order], cls[order], idxv[order], dest[order]
        core_g = [[None] * NCLS for _ in range(ncell)]
        wk = w_s * NCLS + c_s
        cuts = np.flatnonzero(np.diff(wk)) + 1
        starts = np.concatenate([[0], cuts])
        ends = np.concatenate([cuts, [wk.size]])
        for s0, e0 in zip(starts, ends):
            core_g[int(w_s[s0])][int(c_s[s0])] = (i_s[s0:e0], d_s[s0:e0])
        grouped.append(core_g)

    # ---- uniform cell sizes (max over cores, ceil to 128) ----
    cell_sz = np.zeros((ncell, NCLS), np.int64)
    for wn in range(ncell):
        for cl in range(NCLS):
            mx = 0
            for c in range(ncores):
                g = grouped[c][wn][cl]
                if g is not None:
                    mx = max(mx, g[0].size)
            cell_sz[wn, cl] = -(-mx // 128) * 128

    # ---- class streams -> calls + per-(cell,cls) chunk ranges ----
    calls = []            # (cls, num_idxs, off16, offch)
    off16 = 0
    offch = 0
    cls_cell_off = {}     # (cell, cls) -> (chunk_start, nchunks)
    for cl in range(NCLS):
        stream_len = int(cell_sz[:, cl].sum())
        if stream_len == 0:
            continue
        pos = 0
        base_ch = offch
        for wn in range(ncell):
            n = int(cell_sz[wn, cl])
            if n:
                cls_cell_off[(wn, cl)] = (base_ch + pos // 128, n // 128)
            pos += n
        left = stream_len
        while left > 0:
            n = min(left, MAXCALL)
            calls.append((cl, n, off16, offch))
            off16 += n // 16
            offch += n // 128
            left -= n

    tot16 = off16
    totch = offch

    # ---- mm-items per cell: (cls, chunk, win_off in {0,1}) with
    # core-uniform straddle structure; each item gets one dst column ----
    # First gather per-core per-(cell,cls) padded dest vectors.
    core_dests = []   # [core][(cell,cls)] -> vd float64 [cell_sz]
    for c in range(ncores):
        dmap = {}
        for wn in range(ncell):
            for cl in range(NCLS):
                n = int(cell_sz[wn, cl])
                if n == 0:
                    continue
                vd = np.full(n, -1.0, np.float64)
                g = grouped[c][wn][cl]
                if g is not None:
                    vd[:g[1].size] = g[1]
                dmap[(wn, cl)] = vd
        core_dests.append(dmap)

    cells2 = [[] for _ in range(ncell)]  # per cell: (cls, items)
    totd = 0
    for wn in range(ncell):
        for cl in range(NCLS):
            if (wn, cl) not in cls_cell_off:
                continue
            ch0, nch = cls_cell_off[(wn, cl)]
            items = []   # (chunk_id, win_off, dcol)
            nwpc = (2 * WIN * 2) // WIN  # windows per cell
            for k in range(nch):
                present = [False] * nwpc
                for c in range(ncores):
                    d = core_dests[c][(wn, cl)][k * 128:(k + 1) * 128]
                    r = d[d >= 0]
                    for woff in range(nwpc):
                        if not present[woff] and r.size and bool(
                            ((r >= woff * WIN) & (r < (woff + 1) * WIN)).any()
                        ):
                            present[woff] = True
                for woff in range(nwpc):
                    if present[woff]:
                        items.append((ch0 + k, woff, totd))
                        totd += 1
            cells2[wn].append((cl, items))

    # ---- per-core packed idx / dest arrays ----
    per_core = []
    for c in range(ncores):
        idx_arr = np.zeros((128, tot16), np.int16)
        dst_arr = np.full((128, totd), -1.0, np.float32)
        # idx: per class stream
        for cl in range(NCLS):
            iv_parts = []
            for wn in range(ncell):
                n = int(cell_sz[wn, cl])
                if n == 0:
                    continue
                vi = np.zeros(n, np.int64)
                g = grouped[c][wn][cl]
                if g is not None:
                    vi[:g[0].size] = g[0]
                iv_parts.append(vi)
            if not iv_parts:
                continue
            iv = np.concatenate(iv_parts)
            pos = 0
            for (ccl, n, o16, och) in calls:
                if ccl != cl:
                    continue
                vi = iv[pos:pos + n]
                pos += n
                blk = vi.astype(np.int16).reshape(n // 16, 16).T
                idx_arr[:, o16:o16 + n // 16] = np.tile(blk, (8, 1))
        # dst: per mm-item column
        for wn in range(ncell):
            for (cl, items) in cells2[wn]:
                ch0, nch = cls_cell_off[(wn, cl)]
                vd = core_dests[c][(wn, cl)]
                for (ch, woff, dcol) in items:
                    d = vd[(ch - ch0) * 128:(ch - ch0 + 1) * 128]
                    col = np.where(
                        (d >= woff * WIN) & (d < (woff + 1) * WIN),
                        d - woff * WIN, -1.0)
                    dst_arr[:, dcol] = col.astype(np.float32)
        per_core.append({"idx": idx_arr, "dst": dst_arr})

    return {
        "calls": calls, "cells2": cells2, "tot16": tot16, "totd": totd,
        "per_core": per_core, "nwin": nwin, "ncell": ncell, "qrows": qrows,
    }


# ---------------------------------------------------------------------------
# Graph
# ---------------------------------------------------------------------------

def build_graph(pl, epc=EPC, ncores=NCORES):
    nwin = pl["nwin"]
    qrows = pl["qrows"]
    calls = pl["calls"]
    cells2 = pl["cells2"]
    tot16 = pl["tot16"]
    totd = pl["totd"]
    trows = ncores * epc

    nc = bacc.Bacc(
        "TRN2", target_bir_lowering=False, debug=False, num_devices=ncores,
        num_swdge_queues=NQ,
    )

    xt_d = nc.dram_tensor("xt", [128, 2, epc], BF16, kind="ExternalInput")
    w_d = nc.dram_tensor("w", [128, 2, H], BF16, kind="ExternalInput")
    bb_d = nc.dram_tensor("bb", [128, H], BF16, kind="ExternalInput")
    oh_d = nc.dram_tensor("oh", [128, H], BF16, kind="ExternalInput")
    io_d = nc.dram_tensor("io", [128, WIN], BF16, kind="ExternalInput")
    idx_d = nc.dram_tensor("idx", [128, tot16], mybir.dt.int16,
                           kind="ExternalInput")
    dst_d = nc.dram_tensor("dst", [128, totd], F32, kind="ExternalInput")
    out_d = nc.dram_tensor("out", [epc, H], F32, kind="ExternalOutput")

    ag_in = nc.dram_tensor("ag_in", [epc, H], BF16)
    table = nc.dram_tensor("table", [trows, H], BF16, addr_space="Shared")

    with tile.TileContext(nc) as tc:
        with (
            tc.tile_pool(name="const", bufs=1) as constp,
            tc.tile_pool(name="xt", bufs=2) as xtp,
            tc.tile_pool(name="ps1", bufs=2, space="PSUM") as ps1p,
            tc.tile_pool(name="tt", bufs=4) as ttp,
            tc.tile_pool(name="g0", bufs=5) as g0p,
            tc.tile_pool(name="g1", bufs=5) as g1p,
            tc.tile_pool(name="g2", bufs=5) as g2p,
            tc.tile_pool(name="g3", bufs=5) as g3p,
            tc.tile_pool(name="g4", bufs=5) as g4p,
            tc.tile_pool(name="S", bufs=12) as sp,
            tc.tile_pool(name="psw", bufs=1, space="PSUM") as pswp,
            tc.tile_pool(name="ot", bufs=3) as otp,
        ):
            gpools = [g0p, g1p, g2p, g3p, g4p]

            # ---- constants ----
            w_t = constp.tile([128, 2, H], BF16)
            nc.sync.dma_start(out=w_t[:, :, :], in_=w_d[:, :, :])
            bb_t = constp.tile([128, H], BF16)
            nc.sync.dma_start(out=bb_t[:, :], in_=bb_d[:, :])
            oh_t = constp.tile([128, H], BF16)
            nc.sync.dma_start(out=oh_t[:, :], in_=oh_d[:, :])
            io_t = constp.tile([128, WIN], BF16)
            nc.sync.dma_start(out=io_t[:, :], in_=io_d[:, :])
            io8_t = constp.tile([128, 8, WIN], BF16)
            for j8 in range(8):
                nc.vector.tensor_copy(out=io8_t[:, j8, :], in_=io_t[:, :])
            idx_t = constp.tile([128, tot16], mybir.dt.int16)
            nc.sync.dma_start(out=idx_t[:, :], in_=idx_d[:, :])
            dst_t = constp.tile([128, totd], F32)
            nc.sync.dma_start(out=dst_t[:, :], in_=dst_d[:, :])
            t_sb = constp.tile([128, epc // 128, H], BF16)

            # ---- Phase 1: t = x @ W -> bf16 -> ag_in (512-edge groups) ----
            for gi in range(epc // 512):
                xt_t = xtp.tile([128, 2, 512], BF16)
                nc.sync.dma_start(
                    out=xt_t[:, :, :], in_=xt_d[:, :, gi * 512:(gi + 1) * 512]
                )
                ps = ps1p.tile([128, 4, H], F32)
                for i in range(4):
                    for cc in range(2):
                        nc.tensor.matmul(
                            out=ps[:, i, :],
                            lhsT=xt_t[:, cc, i * 128:(i + 1) * 128],
                            rhs=w_t[:, cc, :],
                            start=(cc == 0), stop=(cc == 1),
                        )
                nc.scalar.activation(
                    out=t_sb[:, gi * 4:(gi + 1) * 4, :], in_=ps[:, :, :],
                    func=mybir.ActivationFunctionType.Copy,
                )
                nc.sync.dma_start(
                    out=ag_in[gi * 512:(gi + 1) * 512, :]
                    .rearrange("(i p) h -> p i h", p=128),
                    in_=t_sb[:, gi * 4:(gi + 1) * 4, :],
                )

            # ---- AllGather ----
            nc.gpsimd.collective_compute(
                "AllGather",
                mybir.AluOpType.bypass,
                replica_groups=[list(range(ncores))],
                ins=[ag_in.ap().opt()],
                outs=[table.ap().opt()],
            )

            # ---- Gathers: local class first (overlaps AG), then remote ----
            gtiles = {}   # call index -> (tile, nch)

            def emit_call(ci, cl, n, o16, q):
                nch = n // 128
                g = gpools[cl].tile([128, MAXCALL // 128, H], BF16)
                if cl == 0:
                    in_ap = ag_in[:, :]
                else:
                    qlo = (cl - 1) * qrows
                    in_ap = table[qlo:qlo + qrows, :]
                nc.gpsimd.dma_gather(
                    out_ap=g[:, 0:nch, :],
                    in_ap=in_ap,
                    idxs_ap=idx_t[:, o16:o16 + n // 16],
                    num_idxs=n,
                    num_idxs_reg=n,
                    elem_size=H,
                    queue_num=q,
                )
                gtiles[ci] = (g, nch)

            # issue order: all local calls, then remote calls round-robin by
            # window progress (calls list is already per-class stream order;
            # interleave the 4 remote classes)
            local_calls = [(i, c) for i, c in enumerate(calls) if c[0] == 0]
            remote_by_cls = {cl: [(i, c) for i, c in enumerate(calls)
                                  if c[0] == cl] for cl in range(1, NCLS)}
            qi = 0
            for ci, (cl, n, o16, och) in local_calls:
                emit_call(ci, cl, n, o16, qi % NQ)
                qi += 1
            # interleave remote classes
            maxlen = max((len(v) for v in remote_by_cls.values()), default=0)
            for j in range(maxlen):
                for cl in range(1, NCLS):
                    lst = remote_by_cls[cl]
                    if j < len(lst):
                        ci, (ccl, n, o16, och) = lst[j]
                        emit_call(ci, ccl, n, o16, qi % NQ)
                        qi += 1

            # chunk -> (call, offset) map
            chunk_loc = {}
            for ci, (cl, n, o16, och) in enumerate(calls):
                for k in range(n // 128):
                    chunk_loc[och + k] = (ci, k)

            # ---- Reduce: wgroup = 4 windows = 2 cells of 256 edges.
            # Each mm-item (chunk, win_off, dcol) targets psum slice
            # j = 2*(cell%2) + win_off; S columns are the dcol order. ----
            for wg in range(nwin // 4):
                psw4 = [pswp.tile([128, H], F32, name=f"pw{_j}")
                        for _j in range(4)]
                # collect mm-items per psum slice j for stop accounting
                tot_mm = [0] * 4
                cell_items = []   # (j, ch, dcol) in emission order
                for (cl, items) in cells2[wg]:
                    for (ch, woff, dcol) in items:
                        tot_mm[woff] += 1
                        cell_items.append((woff, ch, dcol))
                for j in range(4):
                    nc.tensor.matmul(
                        out=psw4[j][:, :], lhsT=oh_t[:, :], rhs=bb_t[:, :],
                        start=True, stop=(tot_mm[j] == 0),
                    )
                done = [0] * 4
                # S-builds batch over consecutive dcols (8 at a time)
                ii = 0
                while ii < len(cell_items):
                    nb = 1
                    while (nb < 8 and ii + nb < len(cell_items)
                           and cell_items[ii + nb][2]
                           == cell_items[ii][2] + nb):
                        nb += 1
                    d0 = cell_items[ii][2]
                    S = sp.tile([128, 8, WIN], BF16)
                    nc.vector.tensor_tensor(
                        out=S[:, 0:nb, :],
                        in0=io8_t[:, 0:nb, :],
                        in1=dst_t[:, d0:d0 + nb, None]
                        .to_broadcast([128, nb, WIN]),
                        op=mybir.AluOpType.is_equal,
                    )
                    for k in range(nb):
                        j, ch, dcol = cell_items[ii + k]
                        ci, off = chunk_loc[ch]
                        g, _ = gtiles[ci]
                        done[j] += 1
                        nc.tensor.matmul(
                            out=psw4[j][:, :], lhsT=S[:, k, :],
                            rhs=g[:, off, :],
                            start=False, stop=(done[j] == tot_mm[j]),
                        )
                    ii += nb
                ot = otp.tile([128, 4, H], F32)
                for j in range(4):
                    nc.vector.tensor_tensor(
                        out=ot[:, j, :], in0=psw4[j][:, :],
                        in1=t_sb[:, wg * 4 + j, :],
                        op=mybir.AluOpType.add,
                    )
                nc.sync.dma_start(
                    out=out_d[wg * 4 * WIN:(wg + 1) * 4 * WIN, :]
                    .rearrange("(j p) h -> p j h", p=128),
                    in_=ot[:, :, :],
                )

    nc.compile()
    return nc


# ---------------------------------------------------------------------------
# Host prep + entry point
# ---------------------------------------------------------------------------

def prep_core(edge_feats, W, b, c, epc=EPC, ncores=NCORES):
    e_full = edge_feats.shape[0]
    epc_raw = e_full // ncores
    lo, hi = c * epc_raw, (c + 1) * epc_raw

    x = np.zeros((epc, F), np.float32)
    x[:epc_raw] = edge_feats[lo:hi]
    # xt [128 f, 2 cc, epc e]: xt[f, cc, e] = x[e, cc*128 + f]
    xt = np.ascontiguousarray(
        x.T.reshape(2, 128, epc).transpose(1, 0, 2)
    ).astype(NP_BF16)
    # w [128 f, 2 cc, H]: w[f, cc, h] = W[cc*128 + f, h]
    w_arr = np.ascontiguousarray(
        W.reshape(2, 128, H).transpose(1, 0, 2)
    ).astype(NP_BF16)
    bb = np.broadcast_to((K + 1.0) * b, (128, H)).astype(NP_BF16)
    oh = np.zeros((128, H), np.float32)
    oh[0, :] = 1.0
    io = np.broadcast_to(np.arange(WIN, dtype=np.float32), (128, WIN))
    return {
        "xt": xt, "w": w_arr,
        "bb": np.ascontiguousarray(bb),
        "oh": np.ascontiguousarray(oh).astype(NP_BF16),
        "io": np.ascontiguousarray(io).astype(NP_BF16),
    }


_CACHE = {}


def _get(neighbors, epc=EPC, ncores=NCORES):
    key = (epc, ncores, hash(neighbors.tobytes()))
    if key not in _CACHE:
        pl = plan(neighbors, epc, ncores)
        nc = build_graph(pl, epc, ncores)
        _CACHE.clear()
        _CACHE[key] = (nc, pl)
    return _CACHE[key]


def make_in_maps(edge_feats, neighbors, W, b, epc=EPC, ncores=NCORES):
    nc, pl = _get(neighbors, epc, ncores)
    in_maps = []
    for c in range(ncores):
        m = prep_core(edge_feats, W, b, c, epc, ncores)
        m["idx"] = pl["per_core"][c]["idx"]
        m["dst"] = pl["per_core"][c]["dst"]
        in_maps.append(m)
    return nc, in_maps


def kernel(edge_feats, neighbors, W, b):
    edge_feats = np.asarray(edge_feats, np.float32)
    neighbors = np.asarray(neighbors, np.int32)
    W = np.asarray(W, np.float32)
    b = np.asarray(b, np.float32)
    e_full = edge_feats.shape[0]
    epc_raw = e_full // NCORES

    nc, in_maps = make_in_maps(edge_feats, neighbors, W, b)
    res = run_bass_kernel_spmd(nc, in_maps, core_ids=list(range(NCORES)))
    shards = [
        np.asarray(res.results[c]["out"][:epc_raw], np.float32)
        for c in range(NCORES)
    ]
    return np.concatenate(shards, axis=0)


# revision 24
# speedup vs baseline: 1.0466x; 1.0466x over previous
"""Distributed Trainium2 kernel for AGGEdgeGraph message passing (v2).

Reference:
    t = edge_feats @ W + b                      # [E, H]
    out[i] = t[i] + sum_k t[neighbors[i, k]]    # [E, H]

Strategy (8 cores, edge-sharded, E/8 = 12500 -> padded EPC=12800/core):
  - Phase 1: per-core matmul t = x @ W (NO bias) -> bf16 -> ag_in DRAM.
  - AllGather ag_in -> table [8*EPC, H].  Gather index ranges are the 4
    table quarters (25600 rows < 32768, int16 dma_gather idx limit).
  - Tokens: for each edge window (128 edges) and source class
    (0 = local/own-shard incl. a "self" token per edge, 1..4 = table
    quarters), the window's tokens form a cell padded to x128.  Class
    streams are chopped into dma_gather calls of <=1024 idx, issued
    round-robin over DMA queues.  Pad tokens gather row 0 and carry
    dest=-1 so their selector row is all-zero; no memsets, no count regs.
  - Reduce: per window, psum[128e,128h] = onehot-row matmul (adds
    (K+1)*b to every edge) + sum over chunks of matmul(lhsT=S, rhs=g)
    where S[tok, e] = (dest[tok] == iota[e]) built on DVE via batched
    tensor_tensor is_equal (8 chunks per op).  Output copied psum->SBUF
    on the Scalar engine, then DMA'd out.
  - SPMD: same graph on all cores; per-core behavior only via input data
    (idx/dest arrays).  Cell sizes are max over cores.
"""

import sys

if "/opt/trn_rl_repo" not in sys.path:
    sys.path.insert(0, "/opt/trn_rl_repo")

import numpy as np
import ml_dtypes

import concourse.bass as bass
import concourse.bacc as bacc
import concourse.mybir as mybir
import concourse.tile as tile
from concourse.bass_utils import run_bass_kernel_spmd

NCORES = 8
F = 256
H = 128
K = 16

E_FULL = 100000
EPC_RAW = E_FULL // NCORES   # 12500
EPC = 12800                  # padded per-core edges (multiple of 128)
WIN = 128                    # edges per psum window
MAXCALL = 1024               # dma_gather idx cap (DGE ring limit)
NQ = 4                       # SWDGE queues used round-robin (ucode max 4)
NCLS = 5                     # 0 = local shard, 1..4 = table quarters

BF16 = mybir.dt.bfloat16
F32 = mybir.dt.float32
NP_BF16 = ml_dtypes.bfloat16


# ---------------------------------------------------------------------------
# Host-side planning
# ---------------------------------------------------------------------------

def plan(neighbors, epc=EPC, ncores=NCORES):
    """Build the shared call/cell structure and per-core packed arrays.

    Returns dict with:
      calls: list of (cls, num_idxs) in issue order (per class, stream order)
      cells: per window list of (cls, chunk_start, nchunks) in PE order
      tot16, totch
      per_core: list of dicts with 'idx' [128, tot16] int16 and
                'dst' [128, totch] f32
    """
    e_full = neighbors.shape[0]
    epc_raw = e_full // ncores
    nwin = epc // WIN
    CW = 2 * WIN                 # cell width in edges (2 windows per cell)
    ncell = epc // CW
    qrows = 2 * epc              # table quarter rows (2 shards per quarter)

    # ---- per-core token lists grouped by (cell, class) ----
    # token: (idx_in_range, dest_in_cell 0..CW-1)
    grouped = []   # [core][cell][cls] -> (idx int64 array, dest int64 array)
    for c in range(ncores):
        nb = np.asarray(neighbors[c * epc_raw:(c + 1) * epc_raw], np.int64)
        v = epc * (nb // epc_raw) + (nb % epc_raw)      # padded-global rows
        e = np.repeat(np.arange(epc_raw, dtype=np.int64), K)
        v = v.reshape(-1)
        w = e // CW
        dest = e % CW
        is_local = (v // epc) == c
        cls = np.where(is_local, 0, 1 + v // qrows)
        idxv = np.where(is_local, v - c * epc, v - (v // qrows) * qrows)
        order = np.lexsort((dest, cls, w))
        w_s, c_s, i_s, d_s = w[order], cls[order], idxv[order], dest[order]
        core_g = [[None] * NCLS for _ in range(ncell)]
        wk = w_s * NCLS + c_s
        cuts = np.flatnonzero(np.diff(wk)) + 1
        starts = np.concatenate([[0], cuts])
        ends = np.concatenate([cuts, [wk.size]])
        for s0, e0 in zip(starts, ends):
            core_g[int(w_s[s0])][int(c_s[s0])] = (i_s[s0:e0], d_s[s0:e0])
        grouped.append(core_g)

    # ---- uniform cell sizes (max over cores, ceil to 128) ----
    cell_sz = np.zeros((ncell, NCLS), np.int64)
    for wn in range(ncell):
        for cl in range(NCLS):
            mx = 0
            for c in range(ncores):
                g = grouped[c][wn][cl]
                if g is not None:
                    mx = max(mx, g[0].size)
            cell_sz[wn, cl] = -(-mx // 128) * 128

    # ---- class streams -> calls + per-(cell,cls) chunk ranges ----
    calls = []            # (cls, num_idxs, off16, offch)
    off16 = 0
    offch = 0
    cls_cell_off = {}     # (cell, cls) -> (chunk_start, nchunks)
    for cl in range(NCLS):
        stream_len = int(cell_sz[:, cl].sum())
        if stream_len == 0:
            continue
        pos = 0
        base_ch = offch
        for wn in range(ncell):
            n = int(cell_sz[wn, cl])
            if n:
                cls_cell_off[(wn, cl)] = (base_ch + pos // 128, n // 128)
            pos += n
        left = stream_len
        while left > 0:
            n = min(left, MAXCALL)
            calls.append((cl, n, off16, offch))
            off16 += n // 16
            offch += n // 128
            left -= n

    tot16 = off16
    totch = offch

    # ---- mm-items per cell: (cls, chunk, win_off in {0,1}) with
    # core-uniform straddle structure; each item gets one dst column ----
    # First gather per-core per-(cell,cls) padded dest vectors.
    core_dests = []   # [core][(cell,cls)] -> vd float64 [cell_sz]
    for c in range(ncores):
        dmap = {}
        for wn in range(ncell):
            for cl in range(NCLS):
                n = int(cell_sz[wn, cl])
                if n == 0:
                    continue
                vd = np.full(n, -1.0, np.float64)
                g = grouped[c][wn][cl]
                if g is not None:
                    vd[:g[1].size] = g[1]
                dmap[(wn, cl)] = vd
        core_dests.append(dmap)

    cells2 = [[] for _ in range(ncell)]  # per cell: (cls, items)
    totd = 0
    for wn in range(ncell):
        for cl in range(NCLS):
            if (wn, cl) not in cls_cell_off:
                continue
            ch0, nch = cls_cell_off[(wn, cl)]
            items = []   # (chunk_id, win_off, dcol)
            for k in range(nch):
                lo = hi = False
                for c in range(ncores):
                    d = core_dests[c][(wn, cl)][k * 128:(k + 1) * 128]
                    r = d[d >= 0]
                    if r.size:
                        lo = lo or bool((r < WIN).any())
                        hi = hi or bool((r >= WIN).any())
                if lo:
                    items.append((ch0 + k, 0, totd))
                    totd += 1
                if hi:
                    items.append((ch0 + k, 1, totd))
                    totd += 1
            cells2[wn].append((cl, items))

    # ---- per-core packed idx / dest arrays ----
    per_core = []
    for c in range(ncores):
        idx_arr = np.zeros((128, tot16), np.int16)
        dst_arr = np.full((128, totd), -1.0, np.float32)
        # idx: per class stream
        for cl in range(NCLS):
            iv_parts = []
            for wn in range(ncell):
                n = int(cell_sz[wn, cl])
                if n == 0:
                    continue
                vi = np.zeros(n, np.int64)
                g = grouped[c][wn][cl]
                if g is not None:
                    vi[:g[0].size] = g[0]
                iv_parts.append(vi)
            if not iv_parts:
                continue
            iv = np.concatenate(iv_parts)
            pos = 0
            for (ccl, n, o16, och) in calls:
                if ccl != cl:
                    continue
                vi = iv[pos:pos + n]
                pos += n
                blk = vi.astype(np.int16).reshape(n // 16, 16).T
                idx_arr[:, o16:o16 + n // 16] = np.tile(blk, (8, 1))
        # dst: per mm-item column
        for wn in range(ncell):
            for (cl, items) in cells2[wn]:
                ch0, nch = cls_cell_off[(wn, cl)]
                vd = core_dests[c][(wn, cl)]
                for (ch, woff, dcol) in items:
                    d = vd[(ch - ch0) * 128:(ch - ch0 + 1) * 128]
                    if woff == 0:
                        col = np.where((d >= 0) & (d < WIN), d, -1.0)
                    else:
                        col = np.where(d >= WIN, d - WIN, -1.0)
                    dst_arr[:, dcol] = col.astype(np.float32)
        per_core.append({"idx": idx_arr, "dst": dst_arr})

    return {
        "calls": calls, "cells2": cells2, "tot16": tot16, "totd": totd,
        "per_core": per_core, "nwin": nwin, "ncell": ncell, "qrows": qrows,
    }


# ---------------------------------------------------------------------------
# Graph
# ---------------------------------------------------------------------------

def build_graph(pl, epc=EPC, ncores=NCORES):
    nwin = pl["nwin"]
    qrows = pl["qrows"]
    calls = pl["calls"]
    cells2 = pl["cells2"]
    tot16 = pl["tot16"]
    totd = pl["totd"]
    trows = ncores * epc

    nc = bacc.Bacc(
        "TRN2", target_bir_lowering=False, debug=False, num_devices=ncores,
        num_swdge_queues=NQ,
    )

    xt_d = nc.dram_tensor("xt", [128, 2, epc], BF16, kind="ExternalInput")
    w_d = nc.dram_tensor("w", [128, 2, H], BF16, kind="ExternalInput")
    bb_d = nc.dram_tensor("bb", [128, H], BF16, kind="ExternalInput")
    oh_d = nc.dram_tensor("oh", [128, H], BF16, kind="ExternalInput")
    io_d = nc.dram_tensor("io", [128, WIN], BF16, kind="ExternalInput")
    idx_d = nc.dram_tensor("idx", [128, tot16], mybir.dt.int16,
                           kind="ExternalInput")
    dst_d = nc.dram_tensor("dst", [128, totd], F32, kind="ExternalInput")
    out_d = nc.dram_tensor("out", [epc, H], F32, kind="ExternalOutput")

    ag_in = nc.dram_tensor("ag_in", [epc, H], BF16)
    table = nc.dram_tensor("table", [trows, H], BF16, addr_space="Shared")

    with tile.TileContext(nc) as tc:
        with (
            tc.tile_pool(name="const", bufs=1) as constp,
            tc.tile_pool(name="xt", bufs=2) as xtp,
            tc.tile_pool(name="ps1", bufs=2, space="PSUM") as ps1p,
            tc.tile_pool(name="tt", bufs=4) as ttp,
            tc.tile_pool(name="g0", bufs=5) as g0p,
            tc.tile_pool(name="g1", bufs=5) as g1p,
            tc.tile_pool(name="g2", bufs=5) as g2p,
            tc.tile_pool(name="g3", bufs=5) as g3p,
            tc.tile_pool(name="g4", bufs=5) as g4p,
            tc.tile_pool(name="S", bufs=12) as sp,
            tc.tile_pool(name="psw", bufs=1, space="PSUM") as pswp,
            tc.tile_pool(name="ot", bufs=3) as otp,
        ):
            gpools = [g0p, g1p, g2p, g3p, g4p]

            # ---- constants ----
            w_t = constp.tile([128, 2, H], BF16)
            nc.sync.dma_start(out=w_t[:, :, :], in_=w_d[:, :, :])
            bb_t = constp.tile([128, H], BF16)
            nc.sync.dma_start(out=bb_t[:, :], in_=bb_d[:, :])
            oh_t = constp.tile([128, H], BF16)
            nc.sync.dma_start(out=oh_t[:, :], in_=oh_d[:, :])
            io_t = constp.tile([128, WIN], BF16)
            nc.sync.dma_start(out=io_t[:, :], in_=io_d[:, :])
            io8_t = constp.tile([128, 8, WIN], BF16)
            for j8 in range(8):
                nc.vector.tensor_copy(out=io8_t[:, j8, :], in_=io_t[:, :])
            idx_t = constp.tile([128, tot16], mybir.dt.int16)
            nc.sync.dma_start(out=idx_t[:, :], in_=idx_d[:, :])
            dst_t = constp.tile([128, totd], F32)
            nc.sync.dma_start(out=dst_t[:, :], in_=dst_d[:, :])
            t_sb = constp.tile([128, epc // 128, H], BF16)

            # ---- Phase 1: t = x @ W -> bf16 -> ag_in (512-edge groups) ----
            for gi in range(epc // 512):
                xt_t = xtp.tile([128, 2, 512], BF16)
                nc.sync.dma_start(
                    out=xt_t[:, :, :], in_=xt_d[:, :, gi * 512:(gi + 1) * 512]
                )
                ps = ps1p.tile([128, 4, H], F32)
                for i in range(4):
                    for cc in range(2):
                        nc.tensor.matmul(
                            out=ps[:, i, :],
                            lhsT=xt_t[:, cc, i * 128:(i + 1) * 128],
                            rhs=w_t[:, cc, :],
                            start=(cc == 0), stop=(cc == 1),
                        )
                nc.scalar.activation(
                    out=t_sb[:, gi * 4:(gi + 1) * 4, :], in_=ps[:, :, :],
                    func=mybir.ActivationFunctionType.Copy,
                )
                nc.sync.dma_start(
                    out=ag_in[gi * 512:(gi + 1) * 512, :]
                    .rearrange("(i p) h -> p i h", p=128),
                    in_=t_sb[:, gi * 4:(gi + 1) * 4, :],
                )

            # ---- AllGather ----
            nc.gpsimd.collective_compute(
                "AllGather",
                mybir.AluOpType.bypass,
                replica_groups=[list(range(ncores))],
                ins=[ag_in.ap().opt()],
                outs=[table.ap().opt()],
            )

            # ---- Gathers: local class first (overlaps AG), then remote ----
            gtiles = {}   # call index -> (tile, nch)

            def emit_call(ci, cl, n, o16, q):
                nch = n // 128
                g = gpools[cl].tile([128, MAXCALL // 128, H], BF16)
                if cl == 0:
                    in_ap = ag_in[:, :]
                else:
                    qlo = (cl - 1) * qrows
                    in_ap = table[qlo:qlo + qrows, :]
                nc.gpsimd.dma_gather(
                    out_ap=g[:, 0:nch, :],
                    in_ap=in_ap,
                    idxs_ap=idx_t[:, o16:o16 + n // 16],
                    num_idxs=n,
                    num_idxs_reg=n,
                    elem_size=H,
                    queue_num=q,
                )
                gtiles[ci] = (g, nch)

            # issue order: all local calls, then remote calls round-robin by
            # window progress (calls list is already per-class stream order;
            # interleave the 4 remote classes)
            local_calls = [(i, c) for i, c in enumerate(calls) if c[0] == 0]
            remote_by_cls = {cl: [(i, c) for i, c in enumerate(calls)
                                  if c[0] == cl] for cl in range(1, NCLS)}
            qi = 0
            for ci, (cl, n, o16, och) in local_calls:
                emit_call(ci, cl, n, o16, qi % NQ)
                qi += 1
            # interleave remote classes
            maxlen = max((len(v) for v in remote_by_cls.values()), default=0)
            for j in range(maxlen):
                for cl in range(1, NCLS):
                    lst = remote_by_cls[cl]
                    if j < len(lst):
                        ci, (ccl, n, o16, och) = lst[j]
                        emit_call(ci, ccl, n, o16, qi % NQ)
                        qi += 1

            # chunk -> (call, offset) map
            chunk_loc = {}
            for ci, (cl, n, o16, och) in enumerate(calls):
                for k in range(n // 128):
                    chunk_loc[och + k] = (ci, k)

            # ---- Reduce: wgroup = 4 windows = 2 cells of 256 edges.
            # Each mm-item (chunk, win_off, dcol) targets psum slice
            # j = 2*(cell%2) + win_off; S columns are the dcol order. ----
            for wg in range(nwin // 4):
                psw4 = [pswp.tile([128, H], F32, name=f"pw{_j}")
                        for _j in range(4)]
                # collect mm-items per psum slice j for stop accounting
                tot_mm = [0] * 4
                cell_items = []   # (j, ch, dcol) in emission order
                for half in range(2):
                    cid = wg * 2 + half
                    for (cl, items) in cells2[cid]:
                        for (ch, woff, dcol) in items:
                            j = 2 * half + woff
                            tot_mm[j] += 1
                            cell_items.append((j, ch, dcol))
                for j in range(4):
                    nc.tensor.matmul(
                        out=psw4[j][:, :], lhsT=oh_t[:, :], rhs=bb_t[:, :],
                        start=True, stop=(tot_mm[j] == 0),
                    )
                done = [0] * 4
                # S-builds batch over consecutive dcols (8 at a time)
                ii = 0
                while ii < len(cell_items):
                    nb = 1
                    while (nb < 8 and ii + nb < len(cell_items)
                           and cell_items[ii + nb][2]
                           == cell_items[ii][2] + nb):
                        nb += 1
                    d0 = cell_items[ii][2]
                    S = sp.tile([128, 8, WIN], BF16)
                    nc.vector.tensor_tensor(
                        out=S[:, 0:nb, :],
                        in0=io8_t[:, 0:nb, :],
                        in1=dst_t[:, d0:d0 + nb, None]
                        .to_broadcast([128, nb, WIN]),
                        op=mybir.AluOpType.is_equal,
                    )
                    for k in range(nb):
                        j, ch, dcol = cell_items[ii + k]
                        ci, off = chunk_loc[ch]
                        g, _ = gtiles[ci]
                        done[j] += 1
                        nc.tensor.matmul(
                            out=psw4[j][:, :], lhsT=S[:, k, :],
                            rhs=g[:, off, :],
                            start=False, stop=(done[j] == tot_mm[j]),
                        )
                    ii += nb
                ot = otp.tile([128, 4, H], F32)
                for j in range(4):
                    nc.vector.tensor_tensor(
                        out=ot[:, j, :], in0=psw4[j][:, :],
                        in1=t_sb[:, wg * 4 + j, :],
                        op=mybir.AluOpType.add,
                    )
                nc.sync.dma_start(
                    out=out_d[wg * 4 * WIN:(wg + 1) * 4 * WIN, :]
                    .rearrange("(j p) h -> p j h", p=128),
                    in_=ot[:, :, :],
                )

    nc.compile()
    return nc


# ---------------------------------------------------------------------------
# Host prep + entry point
# ---------------------------------------------------------------------------

def prep_core(edge_feats, W, b, c, epc=EPC, ncores=NCORES):
    e_full = edge_feats.shape[0]
    epc_raw = e_full // ncores
    lo, hi = c * epc_raw, (c + 1) * epc_raw

    x = np.zeros((epc, F), np.float32)
    x[:epc_raw] = edge_feats[lo:hi]
    # xt [128 f, 2 cc, epc e]: xt[f, cc, e] = x[e, cc*128 + f]
    xt = np.ascontiguousarray(
        x.T.reshape(2, 128, epc).transpose(1, 0, 2)
    ).astype(NP_BF16)
    # w [128 f, 2 cc, H]: w[f, cc, h] = W[cc*128 + f, h]
    w_arr = np.ascontiguousarray(
        W.reshape(2, 128, H).transpose(1, 0, 2)
    ).astype(NP_BF16)
    bb = np.broadcast_to((K + 1.0) * b, (128, H)).astype(NP_BF16)
    oh = np.zeros((128, H), np.float32)
    oh[0, :] = 1.0
    io = np.broadcast_to(np.arange(WIN, dtype=np.float32), (128, WIN))
    return {
        "xt": xt, "w": w_arr,
        "bb": np.ascontiguousarray(bb),
        "oh": np.ascontiguousarray(oh).astype(NP_BF16),
        "io": np.ascontiguousarray(io).astype(NP_BF16),
    }


_CACHE = {}


def _get(neighbors, epc=EPC, ncores=NCORES):
    key = (epc, ncores, hash(neighbors.tobytes()))
    if key not in _CACHE:
        pl = plan(neighbors, epc, ncores)
        nc = build_graph(pl, epc, ncores)
        _CACHE.clear()
        _CACHE[key] = (nc, pl)
    return _CACHE[key]


def make_in_maps(edge_feats, neighbors, W, b, epc=EPC, ncores=NCORES):
    nc, pl = _get(neighbors, epc, ncores)
    in_maps = []
    for c in range(ncores):
        m = prep_core(edge_feats, W, b, c, epc, ncores)
        m["idx"] = pl["per_core"][c]["idx"]
        m["dst"] = pl["per_core"][c]["dst"]
        in_maps.append(m)
    return nc, in_maps


def kernel(edge_feats, neighbors, W, b):
    edge_feats = np.asarray(edge_feats, np.float32)
    neighbors = np.asarray(neighbors, np.int32)
    W = np.asarray(W, np.float32)
    b = np.asarray(b, np.float32)
    e_full = edge_feats.shape[0]
    epc_raw = e_full // NCORES

    nc, in_maps = make_in_maps(edge_feats, neighbors, W, b)
    res = run_bass_kernel_spmd(nc, in_maps, core_ids=list(range(NCORES)))
    shards = [
        np.asarray(res.results[c]["out"][:epc_raw], np.float32)
        for c in range(NCORES)
    ]
    return np.concatenate(shards, axis=0)
